# revision 18
# baseline (speedup 1.0000x reference)
"""Trainium2 Bass kernel: single-head causal self-attention (fp16 + pairwise K/V dedup).

Reference computation (per batch b):
    Q = x @ Wq ; K = x @ Wk ; V = x @ Wv          (x: [S, D])
    S_sc = Q @ K^T / sqrt(D), causal masked
    out  = softmax(S_sc) @ V

Sharding: 8 cores, 4 batches -> core c handles batch b = c//2 and query
half h = c%2 (1024 interleaved query rows). K/V projections are
DEDUPLICATED across the pair: core h computes K^T and V only for keys
[h*1024, (h+1)*1024), and the halves are exchanged with pairwise
AllGather collectives ([[0,1],[2,3],[4,5],[6,7]]) through DRAM bounce
buffers. The program stays core-uniform: gathered outputs are written
back over the FULL K^T/V SBUF tiles (own half is overwritten with
identical data), so no instruction depends on h.

Performance notes:
  - All matmul operands fp16 (host-converted); PSUM/softmax math fp32;
    output DMA'd fp16 and widened on host.
  - Host pre-tiles every bulk input to [128, n] partition-major layout so
    each DMA moves 8KB contiguous per partition; weights are
    output-half-chunked and x key-chunked so the FIRST matmul group needs
    only 2MB of DMA (split across two queues).
  - Total DMA is ~32MB/core against a shared ~200GB/s fabric, so the
    schedule is bandwidth-aware: only wk/xin move at t=0; wv/wq/xq are
    deferred with REAL write-after-read gates (a scalar Copy that reads
    both the prefetch target and an early K-proj PSUM tile) because the
    tile scheduler reorders same-engine instructions.
  - K^T and V are each exchanged in TWO chunk collectives (4 total,
    pipelined on the cc fabric) and gathered back in PV/score consumption
    order, so attention never waits long on a 2MB monolith.
  - Scores computed TRANSPOSED: S^T[k, q] = sum_d K^T[d,k]*Q^T[d,q]; the
    softmax k-reduction is an N=2 ones-matmul riding the same stationary
    P^T tiles as the P@V matmuls. No max-subtraction (scores ~ N(0,1)).
  - Causal handling: k-tiles beyond each position's extent are skipped at
    compile time (core-uniform bounds); the last two k-tiles of each
    strip are computed for the second q-column only (N=128); only
    diagonal-crossing k-tiles get the (q >= k) multiplicative mask, fully
    kept tiles take exp() straight into P^T.
"""

import sys

try:
    import concourse.bass as bass  # noqa: F401
except ImportError:
    sys.path.insert(0, "/opt/trn_rl_repo")

import numpy as np

import concourse.bass as bass
import concourse.tile as tile
from concourse import bacc, mybir
from concourse.bass_utils import run_bass_kernel_spmd

B, S, D = 4, 2048, 1024
NQ = 1024  # query rows per core
HK = 1024  # keys projected per core (half of S)
P = 128
DT = D // P  # 8 d tiles
KT = S // P  # 16 k tiles
W = 256  # q-strip width
NSTRIP = NQ // W  # 4 strips
F32 = mybir.dt.float32
F16 = mybir.dt.float16
SCALE = 1.0 / np.sqrt(np.float32(D))  # 0.03125
GROUPS = [[0, 1], [2, 3], [4, 5], [6, 7]]
Exp = None  # set below
_NC_CACHE = {}


def build_nc():
    nc = bacc.Bacc(None, target_bir_lowering=False, num_devices=8)
    ACT = mybir.ActivationFunctionType
    # bulk inputs partition-major [128, n]; W output-half-, x key-chunked
    xkvT = nc.dram_tensor("xkvT", [P, 2 * DT * 512], F16, kind="ExternalInput")
    xqT = nc.dram_tensor("xqT", [P, DT * NQ], F16, kind="ExternalInput")
    qg = nc.dram_tensor("qg", [NQ], F32, kind="ExternalInput")
    wq_d = nc.dram_tensor("Wq", [P, 2 * DT * 512], F16, kind="ExternalInput")
    wk_d = nc.dram_tensor("Wk", [P, 2 * DT * 512], F16, kind="ExternalInput")
    wv_d = nc.dram_tensor("Wv", [P, 2 * DT * 512], F16, kind="ExternalInput")
    out_d = nc.dram_tensor("out", [NQ, D], F16, kind="ExternalOutput")
    # collective bounce buffers, one per key-chunk, partition-major
    kag_in = [nc.dram_tensor(f"kag_in{i}", [P, DT * 512], F16) for i in range(2)]
    kag_out = [nc.dram_tensor(f"kag_out{i}", [2 * P, DT * 512], F16) for i in range(2)]
    vag_in = [nc.dram_tensor(f"vag_in{i}", [P, 4 * D], F16) for i in range(2)]
    warm_in = nc.dram_tensor("cc_warm_in", [P, 64], F16)
    warm_out = nc.dram_tensor("cc_warm_out", [2 * P, 64], F16)
    vag_out = [nc.dram_tensor(f"vag_out{i}", [2 * P, 4 * D], F16) for i in range(2)]

    with tile.TileContext(nc) as tc:
        with (
            tc.tile_pool(name="persist", bufs=1) as persist,
            tc.tile_pool(name="misc", bufs=1) as misc,
        ):
            # Persistent attention operands. kT: [half][chunk][d][s-in-chunk]
            kT = persist.tile([P, 2, 2, DT, 512], F16, tag="kT")
            vsb = persist.tile([P, KT, D], F16, tag="vsb")  # V, full
            xq = persist.tile([P, DT, NQ], F16, tag="xq")
            wq = persist.tile([P, 2, DT, 512], F16, tag="wq")
            qgrid = persist.tile([P, NSTRIP, W], F32, tag="qgrid")
            pTall = persist.tile([P, NSTRIP, KT, W], F16, tag="pTall")

            _qgrid_emit = []
            for qs in range(NSTRIP):
                qg_sl = qg[qs * W : (qs + 1) * W]
                _qgrid_emit.append((qgrid[:, qs, :], bass.AP(
                    tensor=qg_sl.tensor,
                    offset=qg_sl.offset,
                    ap=[[0, P]] + list(qg_sl.ap),
                )))

            # Constants: ones column (N=2), per-partition k index vectors
            ones_f = misc.tile([P, 2], F32, tag="ones_f")
            nc.vector.memset(ones_f, 1.0)
            ones = misc.tile([P, 2], F16, tag="ones")
            nc.vector.tensor_copy(ones, ones_f)
            pvec_i = misc.tile([P, 1], mybir.dt.int32, tag="pvec_i")
            nc.gpsimd.iota(pvec_i, pattern=[[0, 1]], base=0, channel_multiplier=1)
            pvec = misc.tile([P, 1], F32, tag="pvec")
            nc.vector.tensor_copy(pvec, pvec_i)
            kvecf = misc.tile([P, KT], F32, tag="kvecf")
            for kt in range(KT):
                nc.vector.tensor_scalar_add(kvecf[:, kt : kt + 1], pvec, float(kt * P))
            gate = misc.tile([P, 2], F32, tag="gate")

            # ---- Phase 1: own-half K^T and V -> bounce DRAM -> AllGather ----
            with (
                tc.tile_pool(name="wkv", bufs=1) as wkvp,
                tc.tile_pool(name="stg", bufs=8) as stgp,
                tc.tile_pool(name="ps1", bufs=8, space="PSUM") as ps1,
            ):
                wk = wkvp.tile([P, 2, DT, 512], F16, tag="wk")
                wv = wkvp.tile([P, 2, DT, 512], F16, tag="wv")
                xin = wkvp.tile([P, 2, DT, 512], F16, tag="xin")
                # t=0 critical DMAs only: x chunk 0 split across sync+scalar,
                # then wk halves on scalar
                nc.sync.dma_start(xin[:, 0, 0:4, :], xkvT[:, 0:2048])
                nc.scalar.dma_start(xin[:, 0, 4:8, :], xkvT[:, 2048:4096])
                nc.scalar.dma_start(wk[:, 0], wk_d[:, 0:4096])
                nc.scalar.dma_start(wk[:, 1], wk_d[:, 4096:8192])

                # Helpers: one K key-chunk / V quarter each, with its
                # collective and gather-ins, so ccK and ccV alternate and
                # every gathered piece lands well before its consumer.
                def k_chunk(ch, cc_ms, in_ms):
                    for do in range(DT):
                        ps = ps1.tile([P, 512], F32, tag="ps1")
                        for di in range(DT):
                            nc.tensor.matmul(
                                ps,
                                wk[:, do // 4, di, (do % 4) * P : (do % 4 + 1) * P],
                                xin[:, ch, di, :],
                                start=(di == 0),
                                stop=(di == DT - 1),
                            )
                        kst = stgp.tile([P, 512], F16, tag="kst")
                        nc.vector.tensor_copy(kst, ps)
                        nc.sync.dma_start(
                            kag_in[ch][:, do * 512 : (do + 1) * 512], kst
                        )
                        if ch == 0 and do == 0:
                            # x chunk 1 queues behind the first stage DMA;
                            # wv/wq/xq deferred with WAR gates read on gpsimd
                            nc.sync.dma_start(xin[:, 1], xkvT[:, 4096:8192])
                            nc.gpsimd.tensor_add(
                                gate, wv[:, :, 0, 0:1], kst[:, 0:2]
                            )
                            nc.gpsimd.tensor_add(
                                gate, wq[:, :, 0, 0:1], kst[:, 0:2]
                            )
                            nc.gpsimd.tensor_add(
                                gate[:, 0:1], xq[:, 0, 0:1], kst[:, 0:1]
                            )
                            nc.gpsimd.dma_start(wv[:, 0], wv_d[:, 0:4096])
                            nc.gpsimd.dma_start(wv[:, 1], wv_d[:, 4096:8192])
                            nc.scalar.dma_start(wq[:, 0], wq_d[:, 0:4096])
                            nc.scalar.dma_start(wq[:, 1], wq_d[:, 4096:8192])
                            nc.gpsimd.dma_start(xq, xqT[:, :])
                    with tc.tile_wait_until(cc_ms):
                        nc.gpsimd.collective_compute(
                            "AllGather",
                            mybir.AluOpType.bypass,
                            replica_groups=GROUPS,
                            ins=[kag_in[ch][:, :].opt()],
                            outs=[kag_out[ch][:, :].opt()],
                        )
                    # chunk-0 pieces on sync, chunk-1 on gpsimd
                    eng = nc.sync if ch == 0 else nc.gpsimd
                    for half in range(2):
                        with tc.tile_wait_until(in_ms + 0.004 * half):
                            eng.dma_start(
                                kT[:, half, ch, :, :],
                                kag_out[ch][half * P : (half + 1) * P, :],
                            )

                def v_quarter(sq, cc_ms, in_ms):
                    for st in range(4):
                        for dh in range(2):
                            ps = ps1.tile([P, 512], F32, tag="ps1")
                            for di in range(DT):
                                nc.tensor.matmul(
                                    ps,
                                    xin[:, sq, di, st * P : (st + 1) * P],
                                    wv[:, dh, di, :],
                                    start=(di == 0),
                                    stop=(di == DT - 1),
                                )
                            vst = stgp.tile([P, 512], F16, tag="vst")
                            nc.scalar.activation(vst, ps, ACT.Copy)
                            nc.scalar.dma_start(
                                vag_in[sq][
                                    :, st * D + dh * 512 : st * D + (dh + 1) * 512
                                ],
                                vst,
                            )
                    with tc.tile_wait_until(cc_ms):
                        nc.gpsimd.collective_compute(
                            "AllGather",
                            mybir.AluOpType.bypass,
                            replica_groups=GROUPS,
                            ins=[vag_in[sq][:, :].opt()],
                            outs=[vag_out[sq][:, :].opt()],
                        )
                    for half in range(2):
                        t0 = half * 8 + sq * 4
                        with tc.tile_wait_until(in_ms + 0.005 * half):
                            nc.sync.dma_start(
                                vsb[:, t0 : t0 + 4, :],
                                vag_out[sq][half * P : (half + 1) * P, :],
                            )

                k_chunk(0, 0.038, 0.050)
                v_quarter(0, 0.056, 0.070)
                k_chunk(1, 0.074, 0.088)
                v_quarter(1, 0.090, 0.104)

            # ---------------- Phase 2: per-q-strip attention ----------------
            with (
                tc.tile_pool(name="strip", bufs=2) as strip,
                tc.tile_pool(name="sm", bufs=4) as sm,
                tc.tile_pool(name="outp", bufs=2) as outp,
                tc.tile_pool(name="ps2", bufs=2, space="PSUM") as ps2p,
                tc.tile_pool(name="psc", bufs=2, space="PSUM") as pscp,
                tc.tile_pool(name="psl", bufs=2, space="PSUM") as pslp,
            ):
                def kslice(kt, di, width_off=0):
                    return kT[
                        :, kt // 8, (kt % 8) // 4, di,
                        (kt % 4) * P : (kt % 4 + 1) * P,
                    ]

                for qs in range(NSTRIP):
                    q0 = qs * W
                    # Q^T strip [d, W]
                    qT = strip.tile([P, DT, W], F16, tag="qT")
                    for do in range(DT):
                        ps = ps2p.tile([P, W], F32, tag="ps2")
                        for di in range(DT):
                            nc.tensor.matmul(
                                ps,
                                wq[:, do // 4, di, (do % 4) * P : (do % 4 + 1) * P],
                                xq[:, di, q0 : q0 + W],
                                start=(di == 0),
                                stop=(di == DT - 1),
                            )
                        nc.vector.tensor_copy(qT[:, do, :], ps)

                    # S^T strip -> exp -> (mask) -> P^T strip.
                    # Strip holds global q-tiles 4qs+h and 4qs+2+h. k-tiles
                    # >= ext_kt are fully masked for both halves: skipped.
                    # k-tiles in [ext0, ext_kt) concern only the second
                    # q-column: computed at half width. k-tiles < 4qs are
                    # fully kept for both halves: no mask needed.
                    ext_kt = 4 * (qs + 1)
                    ext0 = ext_kt - 2  # first q-column extent (= 4qs+2)
                    for kt in range(ext_kt):
                        wo = 0 if kt < ext0 else P  # half-width for the tail
                        ps = ps2p.tile([P, W], F32, tag="ps2")
                        for di in range(DT):
                            nc.tensor.matmul(
                                ps[:, wo:W],
                                kslice(kt, di),
                                qT[:, di, wo:W],
                                start=(di == 0),
                                stop=(di == DT - 1),
                            )
                        if kt < 4 * qs:
                            nc.scalar.activation(
                                pTall[:, qs, kt, :], ps, ACT.Exp,
                                scale=float(SCALE),
                            )
                        else:
                            et = sm.tile([P, W], F32, tag="et")
                            nc.scalar.activation(
                                et[:, wo:W], ps[:, wo:W], ACT.Exp,
                                scale=float(SCALE),
                            )
                            mt = sm.tile([P, W], F32, tag="mt")
                            nc.vector.tensor_scalar(
                                mt[:, wo:W],
                                qgrid[:, qs, wo:W],
                                kvecf[:, kt : kt + 1],
                                None,
                                op0=mybir.AluOpType.is_ge,
                            )
                            nc.vector.tensor_mul(
                                pTall[:, qs, kt, wo:W], et[:, wo:W], mt[:, wo:W]
                            )

                # PV phase, after all scores: by now the V collectives have
                # landed, so this runs stall-free. context = P^T.T @ V with
                # row-sums l via an N=2 ones-matmul on the same stationary.
                for qs in range(NSTRIP):
                    q0 = qs * W
                    ext_kt = 4 * (qs + 1)
                    ncq = W // P
                    cps = [
                        pscp.tile([P, D], F32, tag="psc", name=f"cps{qs}_{i}")
                        for i in range(ncq)
                    ]
                    lps = [
                        pslp.tile([P, 2], F32, tag="psl", name=f"lps{qs}_{i}")
                        for i in range(ncq)
                    ]
                    for kt in range(ext_kt):
                        for qt in range(ncq):
                            ej = 2 * (qs * ncq + qt) + 2  # this position's extent
                            if kt >= ej:
                                continue
                            lhs = pTall[:, qs, kt, qt * P : (qt + 1) * P]
                            nc.tensor.matmul(
                                cps[qt][:, 0:512],
                                lhs,
                                vsb[:, kt, 0:512],
                                start=(kt == 0),
                                stop=(kt == ej - 1),
                            )
                            nc.tensor.matmul(
                                cps[qt][:, 512:1024],
                                lhs,
                                vsb[:, kt, 512:1024],
                                start=(kt == 0),
                                stop=(kt == ej - 1),
                            )
                            nc.tensor.matmul(
                                lps[qt],
                                lhs,
                                ones,
                                start=(kt == 0),
                                stop=(kt == ej - 1),
                            )
                    for qt in range(ncq):
                        qrow = q0 + qt * P
                        rt = sm.tile([P, 1], F32, tag="rt")
                        nc.vector.reciprocal(rt, lps[qt][:, 0:1])
                        ot = outp.tile([P, D], F16, tag="ot")
                        nc.vector.tensor_scalar_mul(ot, cps[qt], rt)
                        eng = nc.sync if qt == 0 else nc.scalar
                        eng.dma_start(out_d[qrow : qrow + P, :], ot)
    nc.compile()
    return nc


def _get_nc(key="f16"):
    if "nc" not in _NC_CACHE:
        _NC_CACHE["nc"] = build_nc()
    return _NC_CACHE["nc"]


def _qsel(h):
    """Query rows for core-half h: global q-tiles h, 2+h, ..., 14+h.

    Position j's tile 2j+h needs only k < (2j+h+1)*128, letting the kernel
    skip fully-masked k-tiles at compile time with a core-uniform program."""
    tiles = np.arange(8) * 2 + h
    return (tiles[:, None] * P + np.arange(P)[None, :]).reshape(-1)


def _ptile(mat):
    """[D, n] -> partition-major [128, (D/128)*n]: row p = concat over
    d-tiles a of mat[a*128+p, :]."""
    d, n = mat.shape
    return np.ascontiguousarray(
        mat.reshape(d // P, P, n).transpose(1, 0, 2).reshape(P, (d // P) * n)
    )


def _ptile_c(mat):
    """[D, 1024] -> [128, 8192] chunk-major: row p = concat over column
    halves c of (concat over d-tiles a of mat[a*128+p, c*512:(c+1)*512])."""
    d, n = mat.shape
    return np.ascontiguousarray(
        mat.reshape(d // P, P, 2, n // 2)
        .transpose(1, 2, 0, 3)
        .reshape(P, d // P * n)
    )


def make_in_maps(x, Wq, Wk, Wv):
    x = np.asarray(x, dtype=np.float32)
    Wq16 = _ptile_c(np.asarray(Wq, dtype=np.float16))
    Wk16 = _ptile_c(np.asarray(Wk, dtype=np.float16))
    Wv16 = _ptile_c(np.asarray(Wv, dtype=np.float16))
    in_maps = []
    for c in range(8):
        b, h = c // 2, c % 2
        qsel = _qsel(h)
        xb16 = x[b].astype(np.float16)
        in_maps.append(
            {
                "xkvT": _ptile_c(np.ascontiguousarray(xb16[h * HK : (h + 1) * HK].T)),
                "xqT": _ptile(np.ascontiguousarray(xb16[qsel].T)),
                "qg": qsel.astype(np.float32),
                "Wq": Wq16,
                "Wk": Wk16,
                "Wv": Wv16,
            }
        )
    return in_maps


def kernel(x, Wq, Wk, Wv, _trace=False, _nc_key="f16"):
    nc = _get_nc(_nc_key)
    in_maps = make_in_maps(x, Wq, Wk, Wv)
    res = run_bass_kernel_spmd(nc, in_maps, core_ids=list(range(8)), trace=_trace)
    out = np.empty((B, S, D), dtype=np.float32)
    for c in range(8):
        b, h = c // 2, c % 2
        out[b, _qsel(h), :] = res.results[c]["out"].astype(np.float32)
    if _trace:
        kernel.last_results = res
    return out


# revision 19
# speedup vs baseline: 1.0236x; 1.0236x over previous
"""Trainium2 Bass kernel: single-head causal self-attention (fp16 + pairwise K/V dedup).

Reference computation (per batch b):
    Q = x @ Wq ; K = x @ Wk ; V = x @ Wv          (x: [S, D])
    S_sc = Q @ K^T / sqrt(D), causal masked
    out  = softmax(S_sc) @ V

Sharding: 8 cores, 4 batches -> core c handles batch b = c//2 and query
half h = c%2 (1024 interleaved query rows). K/V projections are
DEDUPLICATED across the pair: core h computes K^T and V only for keys
[h*1024, (h+1)*1024), and the halves are exchanged with pairwise
AllGather collectives ([[0,1],[2,3],[4,5],[6,7]]) through DRAM bounce
buffers. The program stays core-uniform: gathered outputs are written
back over the FULL K^T/V SBUF tiles (own half is overwritten with
identical data), so no instruction depends on h.

Performance notes:
  - All matmul operands fp16 (host-converted); PSUM/softmax math fp32;
    output DMA'd fp16 and widened on host.
  - Host pre-tiles every bulk input to [128, n] partition-major layout so
    each DMA moves 8KB contiguous per partition; weights are
    output-half-chunked and x key-chunked so the FIRST matmul group needs
    only 2MB of DMA (split across two queues).
  - Total DMA is ~32MB/core against a shared ~200GB/s fabric, so the
    schedule is bandwidth-aware: only wk/xin move at t=0; wv/wq/xq are
    deferred with REAL write-after-read gates (a scalar Copy that reads
    both the prefetch target and an early K-proj PSUM tile) because the
    tile scheduler reorders same-engine instructions.
  - K^T and V are each exchanged in TWO chunk collectives (4 total,
    pipelined on the cc fabric) and gathered back in PV/score consumption
    order, so attention never waits long on a 2MB monolith.
  - Scores computed TRANSPOSED: S^T[k, q] = sum_d K^T[d,k]*Q^T[d,q]; the
    softmax k-reduction is an N=2 ones-matmul riding the same stationary
    P^T tiles as the P@V matmuls. No max-subtraction (scores ~ N(0,1)).
  - Causal handling: k-tiles beyond each position's extent are skipped at
    compile time (core-uniform bounds); the last two k-tiles of each
    strip are computed for the second q-column only (N=128); only
    diagonal-crossing k-tiles get the (q >= k) multiplicative mask, fully
    kept tiles take exp() straight into P^T.
"""

import sys

try:
    import concourse.bass as bass  # noqa: F401
except ImportError:
    sys.path.insert(0, "/opt/trn_rl_repo")

import numpy as np

import concourse.bass as bass
import concourse.tile as tile
from concourse import bacc, mybir
from concourse.bass_utils import run_bass_kernel_spmd

B, S, D = 4, 2048, 1024
NQ = 1024  # query rows per core
HK = 1024  # keys projected per core (half of S)
P = 128
DT = D // P  # 8 d tiles
KT = S // P  # 16 k tiles
W = 256  # q-strip width
NSTRIP = NQ // W  # 4 strips
F32 = mybir.dt.float32
F16 = mybir.dt.float16
SCALE = 1.0 / np.sqrt(np.float32(D))  # 0.03125
GROUPS = [[0, 1], [2, 3], [4, 5], [6, 7]]
Exp = None  # set below
_NC_CACHE = {}


def build_nc():
    nc = bacc.Bacc(None, target_bir_lowering=False, num_devices=8)
    ACT = mybir.ActivationFunctionType
    # bulk inputs partition-major [128, n]; W output-half-, x key-chunked
    xkvT = nc.dram_tensor("xkvT", [P, 2 * DT * 512], F16, kind="ExternalInput")
    xqT = nc.dram_tensor("xqT", [P, DT * NQ], F16, kind="ExternalInput")
    qg = nc.dram_tensor("qg", [NQ], F32, kind="ExternalInput")
    wq_d = nc.dram_tensor("Wq", [P, 2 * DT * 512], F16, kind="ExternalInput")
    wk_d = nc.dram_tensor("Wk", [P, 2 * DT * 512], F16, kind="ExternalInput")
    wv_d = nc.dram_tensor("Wv", [P, 2 * DT * 512], F16, kind="ExternalInput")
    out_d = nc.dram_tensor("out", [NQ, D], F16, kind="ExternalOutput")
    # collective bounce buffers, one per key-chunk, partition-major
    kag_in = [nc.dram_tensor(f"kag_in{i}", [P, DT * 512], F16) for i in range(2)]
    kag_out = [nc.dram_tensor(f"kag_out{i}", [2 * P, DT * 512], F16) for i in range(2)]
    vag_in = [nc.dram_tensor(f"vag_in{i}", [P, 4 * D], F16) for i in range(2)]
    warm_in = nc.dram_tensor("cc_warm_in", [P, 64], F16)
    warm_out = nc.dram_tensor("cc_warm_out", [2 * P, 64], F16)
    vag_out = [nc.dram_tensor(f"vag_out{i}", [2 * P, 4 * D], F16) for i in range(2)]

    with tile.TileContext(nc) as tc:
        with (
            tc.tile_pool(name="persist", bufs=1) as persist,
            tc.tile_pool(name="misc", bufs=1) as misc,
        ):
            # Persistent attention operands. kT: [half][chunk][d][s-in-chunk]
            kT = persist.tile([P, 2, 2, DT, 512], F16, tag="kT")
            vsb = persist.tile([P, KT, D], F16, tag="vsb")  # V, full
            xq = persist.tile([P, DT, NQ], F16, tag="xq")
            wq = persist.tile([P, 2, DT, 512], F16, tag="wq")
            qgrid = persist.tile([P, NSTRIP, W], F32, tag="qgrid")
            pTall = persist.tile([P, NSTRIP, KT, W], F16, tag="pTall")

            _qgrid_emit = []
            for qs in range(NSTRIP):
                qg_sl = qg[qs * W : (qs + 1) * W]
                _qgrid_emit.append((qgrid[:, qs, :], bass.AP(
                    tensor=qg_sl.tensor,
                    offset=qg_sl.offset,
                    ap=[[0, P]] + list(qg_sl.ap),
                )))

            # Constants: ones column (N=2), per-partition k index vectors
            ones_f = misc.tile([P, 2], F32, tag="ones_f")
            nc.vector.memset(ones_f, 1.0)
            ones = misc.tile([P, 2], F16, tag="ones")
            nc.vector.tensor_copy(ones, ones_f)
            pvec_i = misc.tile([P, 1], mybir.dt.int32, tag="pvec_i")
            nc.gpsimd.iota(pvec_i, pattern=[[0, 1]], base=0, channel_multiplier=1)
            pvec = misc.tile([P, 1], F32, tag="pvec")
            nc.vector.tensor_copy(pvec, pvec_i)
            kvecf = misc.tile([P, KT], F32, tag="kvecf")
            for kt in range(KT):
                nc.vector.tensor_scalar_add(kvecf[:, kt : kt + 1], pvec, float(kt * P))
            gate = misc.tile([P, 2], F32, tag="gate")

            # ---- Phase 1: own-half K^T and V -> bounce DRAM -> AllGather ----
            with (
                tc.tile_pool(name="wkv", bufs=1) as wkvp,
                tc.tile_pool(name="stg", bufs=8) as stgp,
                tc.tile_pool(name="ps1", bufs=8, space="PSUM") as ps1,
            ):
                wk = wkvp.tile([P, 2, DT, 512], F16, tag="wk")
                wv = wkvp.tile([P, 2, DT, 512], F16, tag="wv")
                xin = wkvp.tile([P, 2, DT, 512], F16, tag="xin")
                # t=0 critical DMAs only: x chunk 0 split across sync+scalar,
                # then wk halves on scalar
                nc.sync.dma_start(xin[:, 0, 0:4, :], xkvT[:, 0:2048])
                nc.scalar.dma_start(xin[:, 0, 4:8, :], xkvT[:, 2048:4096])
                nc.scalar.dma_start(wk[:, 0], wk_d[:, 0:4096])
                nc.scalar.dma_start(wk[:, 1], wk_d[:, 4096:8192])

                # K^T own half: out[d_out, s_own] accumulated over d_in;
                # PSUM drains (vector) -> stage tiles -> kag_in[ch].
                # Deferred prefetches are gated behind early K groups with
                # REAL WAR deps: a scalar Copy reads (target, ps) before the
                # prefetch DMA may write the target.
                for ch in range(2):
                    for do in range(DT):
                        g = ch * DT + do
                        ps = ps1.tile([P, 512], F32, tag="ps1")
                        for di in range(DT):
                            nc.tensor.matmul(
                                ps,
                                wk[:, do // 4, di, (do % 4) * P : (do % 4 + 1) * P],
                                xin[:, ch, di, :],
                                start=(di == 0),
                                stop=(di == DT - 1),
                            )
                        kst = stgp.tile([P, 512], F16, tag="kst")
                        nc.vector.tensor_copy(kst, ps)
                        nc.sync.dma_start(
                            kag_in[ch][:, do * 512 : (do + 1) * 512], kst
                        )
                        if g == 0:
                            # x chunk 1 queues behind the first stage DMA
                            nc.sync.dma_start(xin[:, 1], xkvT[:, 4096:8192])
                            # wv gated on group 0 (WAR via kst read);
                            # transfers on gpsimd queue
                            nc.vector.tensor_add(
                                gate, wv[:, :, 0, 0:1], kst[:, 0:2]
                            )
                            nc.gpsimd.dma_start(wv[:, 0], wv_d[:, 0:4096])
                            nc.gpsimd.dma_start(wv[:, 1], wv_d[:, 4096:8192])
                        elif g == 2:
                            nc.vector.tensor_add(
                                gate, wq[:, :, 0, 0:1], kst[:, 0:2]
                            )
                            nc.scalar.dma_start(wq[:, 0], wq_d[:, 0:4096])
                            nc.scalar.dma_start(wq[:, 1], wq_d[:, 4096:8192])
                        elif g == 4:
                            nc.vector.tensor_add(
                                gate[:, 0:1], xq[:, 0, 0:1], kst[:, 0:1]
                            )
                            nc.scalar.dma_start(xq, xqT[:, :])
                    # exchange this key-chunk as soon as its stages land.
                    # tile_wait_until teaches the scheduler's sim the real
                    # collective latency so it orders the PE stream sanely.
                    with tc.tile_wait_until(0.040 if ch == 0 else 0.054):
                        nc.gpsimd.collective_compute(
                            "AllGather",
                            mybir.AluOpType.bypass,
                            replica_groups=GROUPS,
                            ins=[kag_in[ch][:, :].opt()],
                            outs=[kag_out[ch][:, :].opt()],
                        )
                # gather K^T back in score-consumption order: chunk-0
                # pieces (tiles 0-3, 8-11) on sync, chunk-1 (4-7, 12-15)
                # on gpsimd so neither queue serializes all four
                for half in range(2):
                    with tc.tile_wait_until(0.050 + 0.004 * half):
                        nc.sync.dma_start(
                            kT[:, half, 0, :, :],
                            kag_out[0][half * P : (half + 1) * P, :],
                        )
                for half in range(2):
                    with tc.tile_wait_until(0.066 + 0.004 * half):
                        nc.gpsimd.dma_start(
                            kT[:, half, 1, :, :],
                            kag_out[1][half * P : (half + 1) * P, :],
                        )

                # V own half in two quarters; each quarter AllGathers as soon
                # as its 8 stage DMAs land. PSUM drains on the scalar engine.
                for sq in range(2):
                    for st in range(4):
                        for dh in range(2):
                            ps = ps1.tile([P, 512], F32, tag="ps1")
                            for di in range(DT):
                                nc.tensor.matmul(
                                    ps,
                                    xin[:, sq, di, st * P : (st + 1) * P],
                                    wv[:, dh, di, :],
                                    start=(di == 0),
                                    stop=(di == DT - 1),
                                )
                            vst = stgp.tile([P, 512], F16, tag="vst")
                            nc.scalar.activation(vst, ps, ACT.Copy)
                            nc.scalar.dma_start(
                                vag_in[sq][
                                    :, st * D + dh * 512 : st * D + (dh + 1) * 512
                                ],
                                vst,
                            )
                    with tc.tile_wait_until(0.072 if sq == 0 else 0.086):
                        nc.gpsimd.collective_compute(
                            "AllGather",
                            mybir.AluOpType.bypass,
                            replica_groups=GROUPS,
                            ins=[vag_in[sq][:, :].opt()],
                            outs=[vag_out[sq][:, :].opt()],
                        )
                # gather V back in PV-consumption order:
                # tiles 0-3 (cc0 s0), 4-7 (cc1 s0), 8-11 (cc0 s1), 12-15
                for i, (sq, half) in enumerate(((0, 0), (1, 0), (0, 1), (1, 1))):
                    t0 = half * 8 + sq * 4
                    eng = nc.sync if half == 0 else nc.scalar
                    with tc.tile_wait_until(0.080 + 0.007 * i):
                        eng.dma_start(
                            vsb[:, t0 : t0 + 4, :],
                            vag_out[sq][half * P : (half + 1) * P, :],
                        )

            # ---------------- Phase 2: per-q-strip attention ----------------
            with (
                tc.tile_pool(name="strip", bufs=2) as strip,
                tc.tile_pool(name="sm", bufs=4) as sm,
                tc.tile_pool(name="outp", bufs=2) as outp,
                tc.tile_pool(name="ps2", bufs=2, space="PSUM") as ps2p,
                tc.tile_pool(name="psc", bufs=2, space="PSUM") as pscp,
                tc.tile_pool(name="psl", bufs=2, space="PSUM") as pslp,
            ):
                def kslice(kt, di, width_off=0):
                    return kT[
                        :, kt // 8, (kt % 8) // 4, di,
                        (kt % 4) * P : (kt % 4 + 1) * P,
                    ]

                for qs in range(NSTRIP):
                    q0 = qs * W
                    # Q^T strip [d, W]
                    qT = strip.tile([P, DT, W], F16, tag="qT")
                    for do in range(DT):
                        ps = ps2p.tile([P, W], F32, tag="ps2")
                        for di in range(DT):
                            nc.tensor.matmul(
                                ps,
                                wq[:, do // 4, di, (do % 4) * P : (do % 4 + 1) * P],
                                xq[:, di, q0 : q0 + W],
                                start=(di == 0),
                                stop=(di == DT - 1),
                            )
                        nc.vector.tensor_copy(qT[:, do, :], ps)

                    # S^T strip -> exp -> (mask) -> P^T strip.
                    # Strip holds global q-tiles 4qs+h and 4qs+2+h. k-tiles
                    # >= ext_kt are fully masked for both halves: skipped.
                    # k-tiles in [ext0, ext_kt) concern only the second
                    # q-column: computed at half width. k-tiles < 4qs are
                    # fully kept for both halves: no mask needed.
                    ext_kt = 4 * (qs + 1)
                    ext0 = ext_kt - 2  # first q-column extent (= 4qs+2)
                    for kt in range(ext_kt):
                        wo = 0 if kt < ext0 else P  # half-width for the tail
                        ps = ps2p.tile([P, W], F32, tag="ps2")
                        for di in range(DT):
                            nc.tensor.matmul(
                                ps[:, wo:W],
                                kslice(kt, di),
                                qT[:, di, wo:W],
                                start=(di == 0),
                                stop=(di == DT - 1),
                            )
                        if kt < 4 * qs:
                            nc.scalar.activation(
                                pTall[:, qs, kt, :], ps, ACT.Exp,
                                scale=float(SCALE),
                            )
                        else:
                            et = sm.tile([P, W], F32, tag="et")
                            nc.scalar.activation(
                                et[:, wo:W], ps[:, wo:W], ACT.Exp,
                                scale=float(SCALE),
                            )
                            mt = sm.tile([P, W], F32, tag="mt")
                            nc.vector.tensor_scalar(
                                mt[:, wo:W],
                                qgrid[:, qs, wo:W],
                                kvecf[:, kt : kt + 1],
                                None,
                                op0=mybir.AluOpType.is_ge,
                            )
                            nc.vector.tensor_mul(
                                pTall[:, qs, kt, wo:W], et[:, wo:W], mt[:, wo:W]
                            )

                # PV phase, after all scores: by now the V collectives have
                # landed, so this runs stall-free. context = P^T.T @ V with
                # row-sums l via an N=2 ones-matmul on the same stationary.
                for qs in range(NSTRIP):
                    q0 = qs * W
                    ext_kt = 4 * (qs + 1)
                    ncq = W // P
                    cps = [
                        pscp.tile([P, D], F32, tag="psc", name=f"cps{qs}_{i}")
                        for i in range(ncq)
                    ]
                    lps = [
                        pslp.tile([P, 2], F32, tag="psl", name=f"lps{qs}_{i}")
                        for i in range(ncq)
                    ]
                    for kt in range(ext_kt):
                        for qt in range(ncq):
                            ej = 2 * (qs * ncq + qt) + 2  # this position's extent
                            if kt >= ej:
                                continue
                            lhs = pTall[:, qs, kt, qt * P : (qt + 1) * P]
                            nc.tensor.matmul(
                                cps[qt][:, 0:512],
                                lhs,
                                vsb[:, kt, 0:512],
                                start=(kt == 0),
                                stop=(kt == ej - 1),
                            )
                            nc.tensor.matmul(
                                cps[qt][:, 512:1024],
                                lhs,
                                vsb[:, kt, 512:1024],
                                start=(kt == 0),
                                stop=(kt == ej - 1),
                            )
                            nc.tensor.matmul(
                                lps[qt],
                                lhs,
                                ones,
                                start=(kt == 0),
                                stop=(kt == ej - 1),
                            )
                    for qt in range(ncq):
                        qrow = q0 + qt * P
                        rt = sm.tile([P, 1], F32, tag="rt")
                        nc.vector.reciprocal(rt, lps[qt][:, 0:1])
                        ot = outp.tile([P, D], F16, tag="ot")
                        nc.vector.tensor_scalar_mul(
                            ot[:, 0:512], cps[qt][:, 0:512], rt
                        )
                        nc.vector.tensor_scalar_mul(
                            ot[:, 512:1024], cps[qt][:, 512:1024], rt
                        )
                        e0 = nc.sync if qt == 0 else nc.scalar
                        e1 = nc.scalar if qt == 0 else nc.sync
                        e0.dma_start(out_d[qrow : qrow + P, 0:512], ot[:, 0:512])
                        e1.dma_start(
                            out_d[qrow : qrow + P, 512:1024], ot[:, 512:1024]
                        )
    nc.compile()
    return nc


def _get_nc(key="f16"):
    if "nc" not in _NC_CACHE:
        _NC_CACHE["nc"] = build_nc()
    return _NC_CACHE["nc"]


def _qsel(h):
    """Query rows for core-half h: global q-tiles h, 2+h, ..., 14+h.

    Position j's tile 2j+h needs only k < (2j+h+1)*128, letting the kernel
    skip fully-masked k-tiles at compile time with a core-uniform program."""
    tiles = np.arange(8) * 2 + h
    return (tiles[:, None] * P + np.arange(P)[None, :]).reshape(-1)


def _ptile(mat):
    """[D, n] -> partition-major [128, (D/128)*n]: row p = concat over
    d-tiles a of mat[a*128+p, :]."""
    d, n = mat.shape
    return np.ascontiguousarray(
        mat.reshape(d // P, P, n).transpose(1, 0, 2).reshape(P, (d // P) * n)
    )


def _ptile_c(mat):
    """[D, 1024] -> [128, 8192] chunk-major: row p = concat over column
    halves c of (concat over d-tiles a of mat[a*128+p, c*512:(c+1)*512])."""
    d, n = mat.shape
    return np.ascontiguousarray(
        mat.reshape(d // P, P, 2, n // 2)
        .transpose(1, 2, 0, 3)
        .reshape(P, d // P * n)
    )


def make_in_maps(x, Wq, Wk, Wv):
    x = np.asarray(x, dtype=np.float32)
    Wq16 = _ptile_c(np.asarray(Wq, dtype=np.float16))
    Wk16 = _ptile_c(np.asarray(Wk, dtype=np.float16))
    Wv16 = _ptile_c(np.asarray(Wv, dtype=np.float16))
    in_maps = []
    for c in range(8):
        b, h = c // 2, c % 2
        qsel = _qsel(h)
        xb16 = x[b].astype(np.float16)
        in_maps.append(
            {
                "xkvT": _ptile_c(np.ascontiguousarray(xb16[h * HK : (h + 1) * HK].T)),
                "xqT": _ptile(np.ascontiguousarray(xb16[qsel].T)),
                "qg": qsel.astype(np.float32),
                "Wq": Wq16,
                "Wk": Wk16,
                "Wv": Wv16,
            }
        )
    return in_maps


def kernel(x, Wq, Wk, Wv, _trace=False, _nc_key="f16"):
    nc = _get_nc(_nc_key)
    in_maps = make_in_maps(x, Wq, Wk, Wv)
    res = run_bass_kernel_spmd(nc, in_maps, core_ids=list(range(8)), trace=_trace)
    out = np.empty((B, S, D), dtype=np.float32)
    for c in range(8):
        b, h = c // 2, c % 2
        out[b, _qsel(h), :] = res.results[c]["out"].astype(np.float32)
    if _trace:
        kernel.last_results = res
    return out


# revision 20
# speedup vs baseline: 1.0501x; 1.0259x over previous
"""Trainium2 Bass kernel: single-head causal self-attention (fp16 + pairwise K/V dedup).

Reference computation (per batch b):
    Q = x @ Wq ; K = x @ Wk ; V = x @ Wv          (x: [S, D])
    S_sc = Q @ K^T / sqrt(D), causal masked
    out  = softmax(S_sc) @ V

Sharding: 8 cores, 4 batches -> core c handles batch b = c//2 and query
half h = c%2 (1024 interleaved query rows). K/V projections are
DEDUPLICATED across the pair: core h computes K^T and V only for keys
[h*1024, (h+1)*1024), and the halves are exchanged with pairwise
AllGather collectives ([[0,1],[2,3],[4,5],[6,7]]) through DRAM bounce
buffers. The program stays core-uniform: gathered outputs are written
back over the FULL K^T/V SBUF tiles (own half is overwritten with
identical data), so no instruction depends on h.

Performance notes:
  - All matmul operands fp16 (host-converted); PSUM/softmax math fp32;
    output DMA'd fp16 and widened on host.
  - Host pre-tiles every bulk input to [128, n] partition-major layout so
    each DMA moves 8KB contiguous per partition; weights are
    output-half-chunked and x key-chunked so the FIRST matmul group needs
    only 2MB of DMA (split across two queues).
  - Total DMA is ~32MB/core against a shared ~200GB/s fabric, so the
    schedule is bandwidth-aware: only wk/xin move at t=0; wv/wq/xq are
    deferred with REAL write-after-read gates (a scalar Copy that reads
    both the prefetch target and an early K-proj PSUM tile) because the
    tile scheduler reorders same-engine instructions.
  - K^T and V are each exchanged in TWO chunk collectives (4 total,
    pipelined on the cc fabric) and gathered back in PV/score consumption
    order, so attention never waits long on a 2MB monolith.
  - Scores computed TRANSPOSED: S^T[k, q] = sum_d K^T[d,k]*Q^T[d,q]; the
    softmax k-reduction is an N=2 ones-matmul riding the same stationary
    P^T tiles as the P@V matmuls. No max-subtraction (scores ~ N(0,1)).
  - Causal handling: k-tiles beyond each position's extent are skipped at
    compile time (core-uniform bounds); the last two k-tiles of each
    strip are computed for the second q-column only (N=128); only
    diagonal-crossing k-tiles get the (q >= k) multiplicative mask, fully
    kept tiles take exp() straight into P^T.
"""

import sys

try:
    import concourse.bass as bass  # noqa: F401
except ImportError:
    sys.path.insert(0, "/opt/trn_rl_repo")

import numpy as np

import concourse.bass as bass
import concourse.tile as tile
from concourse import bacc, mybir
from concourse.bass_utils import run_bass_kernel_spmd

B, S, D = 4, 2048, 1024
NQ = 1024  # query rows per core
HK = 1024  # keys projected per core (half of S)
P = 128
DT = D // P  # 8 d tiles
KT = S // P  # 16 k tiles
W = 256  # q-strip width
NSTRIP = NQ // W  # 4 strips
F32 = mybir.dt.float32
F16 = mybir.dt.float16
SCALE = 1.0 / np.sqrt(np.float32(D))  # 0.03125
GROUPS = [[0, 1], [2, 3], [4, 5], [6, 7]]
Exp = None  # set below
_NC_CACHE = {}


def build_nc():
    nc = bacc.Bacc(None, target_bir_lowering=False, num_devices=8)
    ACT = mybir.ActivationFunctionType
    # bulk inputs partition-major [128, n]; W output-half-, x key-chunked
    xkvT = nc.dram_tensor("xkvT", [P, 2 * DT * 512], F16, kind="ExternalInput")
    xqT = nc.dram_tensor("xqT", [P, DT * NQ], F16, kind="ExternalInput")
    qg = nc.dram_tensor("qg", [NQ], F32, kind="ExternalInput")
    wq_d = nc.dram_tensor("Wq", [P, 2 * DT * 512], F16, kind="ExternalInput")
    wk_d = nc.dram_tensor("Wk", [P, 2 * DT * 512], F16, kind="ExternalInput")
    wv_d = nc.dram_tensor("Wv", [P, 2 * DT * 512], F16, kind="ExternalInput")
    out_d = nc.dram_tensor("out", [NQ, D], F16, kind="ExternalOutput")
    # collective bounce buffers, one per key-chunk, partition-major
    kag_in = [nc.dram_tensor(f"kag_in{i}", [P, DT * 512], F16) for i in range(2)]
    kag_out = [nc.dram_tensor(f"kag_out{i}", [2 * P, DT * 512], F16) for i in range(2)]
    vag_in = [nc.dram_tensor(f"vag_in{i}", [P, 4 * D], F16) for i in range(2)]
    warm_in = nc.dram_tensor("cc_warm_in", [P, 64], F16)
    warm_out = nc.dram_tensor("cc_warm_out", [2 * P, 64], F16)
    vag_out = [nc.dram_tensor(f"vag_out{i}", [2 * P, 4 * D], F16) for i in range(2)]

    with tile.TileContext(nc) as tc:
        with (
            tc.tile_pool(name="persist", bufs=1) as persist,
            tc.tile_pool(name="misc", bufs=1) as misc,
        ):
            # Persistent attention operands. kT: [half][chunk][d][s-in-chunk]
            kT = persist.tile([P, 2, 2, DT, 512], F16, tag="kT")
            vsb = persist.tile([P, KT, D], F16, tag="vsb")  # V, full
            xq = persist.tile([P, DT, NQ], F16, tag="xq")
            wq = persist.tile([P, 2, DT, 512], F16, tag="wq")
            qgrid = persist.tile([P, NSTRIP, W], F32, tag="qgrid")
            pTall = persist.tile([P, NSTRIP, KT, W], F16, tag="pTall")

            _qgrid_emit = []
            for qs in range(NSTRIP):
                qg_sl = qg[qs * W : (qs + 1) * W]
                _qgrid_emit.append((qgrid[:, qs, :], bass.AP(
                    tensor=qg_sl.tensor,
                    offset=qg_sl.offset,
                    ap=[[0, P]] + list(qg_sl.ap),
                )))

            # Constants: ones column (N=2), per-partition k index vectors
            ones_f = misc.tile([P, 2], F32, tag="ones_f")
            nc.vector.memset(ones_f, 1.0)
            ones = misc.tile([P, 2], F16, tag="ones")
            nc.vector.tensor_copy(ones, ones_f)
            pvec_i = misc.tile([P, 1], mybir.dt.int32, tag="pvec_i")
            nc.gpsimd.iota(pvec_i, pattern=[[0, 1]], base=0, channel_multiplier=1)
            pvec = misc.tile([P, 1], F32, tag="pvec")
            nc.vector.tensor_copy(pvec, pvec_i)
            kvecf = misc.tile([P, KT], F32, tag="kvecf")
            for kt in range(KT):
                nc.vector.tensor_scalar_add(kvecf[:, kt : kt + 1], pvec, float(kt * P))
            gate = misc.tile([P, 2], F32, tag="gate")

            # ---- Phase 1: own-half K^T and V -> bounce DRAM -> AllGather ----
            with (
                tc.tile_pool(name="wkv", bufs=1) as wkvp,
                tc.tile_pool(name="stg", bufs=8) as stgp,
                tc.tile_pool(name="ps1", bufs=8, space="PSUM") as ps1,
            ):
                wk = wkvp.tile([P, 2, DT, 512], F16, tag="wk")
                wv = wkvp.tile([P, 2, DT, 512], F16, tag="wv")
                xin = wkvp.tile([P, 2, DT, 512], F16, tag="xin")
                # t=0 critical DMAs only: x chunk 0 split across sync+scalar,
                # then wk halves on scalar
                nc.sync.dma_start(xin[:, 0, 0:4, :], xkvT[:, 0:2048])
                nc.scalar.dma_start(xin[:, 0, 4:8, :], xkvT[:, 2048:4096])
                nc.scalar.dma_start(wk[:, 0], wk_d[:, 0:4096])
                nc.scalar.dma_start(wk[:, 1], wk_d[:, 4096:8192])

                # K^T own half: out[d_out, s_own] accumulated over d_in;
                # PSUM drains (vector) -> stage tiles -> kag_in[ch].
                # Deferred prefetches are gated behind early K groups with
                # REAL WAR deps: a scalar Copy reads (target, ps) before the
                # prefetch DMA may write the target.
                for ch in range(2):
                    for do in range(DT):
                        g = ch * DT + do
                        ps = ps1.tile([P, 512], F32, tag="ps1")
                        for di in range(DT):
                            nc.tensor.matmul(
                                ps,
                                wk[:, do // 4, di, (do % 4) * P : (do % 4 + 1) * P],
                                xin[:, ch, di, :],
                                start=(di == 0),
                                stop=(di == DT - 1),
                            )
                        kst = stgp.tile([P, 512], F16, tag="kst")
                        nc.vector.tensor_copy(kst, ps)
                        nc.sync.dma_start(
                            kag_in[ch][:, do * 512 : (do + 1) * 512], kst
                        )
                        if g == 0:
                            # x chunk 1 queues behind the first stage DMA
                            nc.sync.dma_start(xin[:, 1], xkvT[:, 4096:8192])
                            # wv gated on group 0 (WAR via kst read);
                            # transfers on gpsimd queue
                            nc.vector.tensor_add(
                                gate, wv[:, :, 0, 0:1], kst[:, 0:2]
                            )
                            nc.gpsimd.dma_start(wv[:, 0], wv_d[:, 0:4096])
                            nc.gpsimd.dma_start(wv[:, 1], wv_d[:, 4096:8192])
                        elif g == 2:
                            nc.vector.tensor_add(
                                gate, wq[:, :, 0, 0:1], kst[:, 0:2]
                            )
                            nc.scalar.dma_start(wq[:, 0], wq_d[:, 0:4096])
                            nc.scalar.dma_start(wq[:, 1], wq_d[:, 4096:8192])
                        elif g == 4:
                            nc.vector.tensor_add(
                                gate[:, 0:1], xq[:, 0, 0:1], kst[:, 0:1]
                            )
                            nc.scalar.dma_start(xq, xqT[:, :])
                    # exchange this key-chunk as soon as its stages land.
                    # tile_wait_until teaches the scheduler's sim the real
                    # collective latency so it orders the PE stream sanely.
                    with tc.tile_wait_until(0.040 if ch == 0 else 0.054):
                        nc.gpsimd.collective_compute(
                            "AllGather",
                            mybir.AluOpType.bypass,
                            replica_groups=GROUPS,
                            ins=[kag_in[ch][:, :].opt()],
                            outs=[kag_out[ch][:, :].opt()],
                        )
                # gather K^T back in score-consumption order: chunk-0
                # pieces (tiles 0-3, 8-11) on sync, chunk-1 (4-7, 12-15)
                # on gpsimd so neither queue serializes all four
                for half in range(2):
                    with tc.tile_wait_until(0.050 + 0.004 * half):
                        nc.sync.dma_start(
                            kT[:, half, 0, :, :],
                            kag_out[0][half * P : (half + 1) * P, :],
                        )
                for half in range(2):
                    with tc.tile_wait_until(0.066 + 0.004 * half):
                        nc.gpsimd.dma_start(
                            kT[:, half, 1, :, :],
                            kag_out[1][half * P : (half + 1) * P, :],
                        )

                # V own half in two quarters; each quarter AllGathers as soon
                # as its 8 stage DMAs land. PSUM drains on the scalar engine.
                for sq in range(2):
                    for st in range(4):
                        for dh in range(2):
                            ps = ps1.tile([P, 512], F32, tag="ps1")
                            for di in range(DT):
                                nc.tensor.matmul(
                                    ps,
                                    xin[:, sq, di, st * P : (st + 1) * P],
                                    wv[:, dh, di, :],
                                    start=(di == 0),
                                    stop=(di == DT - 1),
                                )
                            vst = stgp.tile([P, 512], F16, tag="vst")
                            nc.scalar.activation(vst, ps, ACT.Copy)
                            nc.scalar.dma_start(
                                vag_in[sq][
                                    :, st * D + dh * 512 : st * D + (dh + 1) * 512
                                ],
                                vst,
                            )
                    with tc.tile_wait_until(0.072 if sq == 0 else 0.086):
                        nc.gpsimd.collective_compute(
                            "AllGather",
                            mybir.AluOpType.bypass,
                            replica_groups=GROUPS,
                            ins=[vag_in[sq][:, :].opt()],
                            outs=[vag_out[sq][:, :].opt()],
                        )
                # gather V back in PV-consumption order:
                # tiles 0-3 (cc0 s0), 4-7 (cc1 s0), 8-11 (cc0 s1), 12-15
                for i, (sq, half) in enumerate(((0, 0), (1, 0), (0, 1), (1, 1))):
                    t0 = half * 8 + sq * 4
                    with tc.tile_wait_until(0.082 + 0.008 * i):
                        nc.sync.dma_start(
                            vsb[:, t0 : t0 + 4, :],
                            vag_out[sq][half * P : (half + 1) * P, :],
                        )

            # ---------------- Phase 2: per-q-strip attention ----------------
            with (
                tc.tile_pool(name="strip", bufs=2) as strip,
                tc.tile_pool(name="sm", bufs=4) as sm,
                tc.tile_pool(name="outp", bufs=2) as outp,
                tc.tile_pool(name="ps2", bufs=2, space="PSUM") as ps2p,
                tc.tile_pool(name="psc", bufs=2, space="PSUM") as pscp,
                tc.tile_pool(name="psl", bufs=2, space="PSUM") as pslp,
            ):
                def kslice(kt, di, width_off=0):
                    return kT[
                        :, kt // 8, (kt % 8) // 4, di,
                        (kt % 4) * P : (kt % 4 + 1) * P,
                    ]

                for qs in range(NSTRIP):
                    q0 = qs * W
                    # Q^T strip [d, W]
                    qT = strip.tile([P, DT, W], F16, tag="qT")
                    for do in range(DT):
                        ps = ps2p.tile([P, W], F32, tag="ps2")
                        for di in range(DT):
                            nc.tensor.matmul(
                                ps,
                                wq[:, do // 4, di, (do % 4) * P : (do % 4 + 1) * P],
                                xq[:, di, q0 : q0 + W],
                                start=(di == 0),
                                stop=(di == DT - 1),
                            )
                        nc.vector.tensor_copy(qT[:, do, :], ps)

                    # S^T strip -> exp -> (mask) -> P^T strip.
                    # Strip holds global q-tiles 4qs+h and 4qs+2+h. k-tiles
                    # >= ext_kt are fully masked for both halves: skipped.
                    # k-tiles in [ext0, ext_kt) concern only the second
                    # q-column: computed at half width. k-tiles < 4qs are
                    # fully kept for both halves: no mask needed.
                    ext_kt = 4 * (qs + 1)
                    ext0 = ext_kt - 2  # first q-column extent (= 4qs+2)
                    for kt in range(ext_kt):
                        wo = 0 if kt < ext0 else P  # half-width for the tail
                        ps = ps2p.tile([P, W], F32, tag="ps2")
                        for di in range(DT):
                            nc.tensor.matmul(
                                ps[:, wo:W],
                                kslice(kt, di),
                                qT[:, di, wo:W],
                                start=(di == 0),
                                stop=(di == DT - 1),
                            )
                        if kt < 4 * qs:
                            nc.scalar.activation(
                                pTall[:, qs, kt, :], ps, ACT.Exp,
                                scale=float(SCALE),
                            )
                        else:
                            et = sm.tile([P, W], F32, tag="et")
                            nc.scalar.activation(
                                et[:, wo:W], ps[:, wo:W], ACT.Exp,
                                scale=float(SCALE),
                            )
                            mt = sm.tile([P, W], F32, tag="mt")
                            nc.vector.tensor_scalar(
                                mt[:, wo:W],
                                qgrid[:, qs, wo:W],
                                kvecf[:, kt : kt + 1],
                                None,
                                op0=mybir.AluOpType.is_ge,
                            )
                            nc.vector.tensor_mul(
                                pTall[:, qs, kt, wo:W], et[:, wo:W], mt[:, wo:W]
                            )

                # PV phase, after all scores: by now the V collectives have
                # landed, so this runs stall-free. context = P^T.T @ V with
                # row-sums l via an N=2 ones-matmul on the same stationary.
                for qs in range(NSTRIP):
                    q0 = qs * W
                    ext_kt = 4 * (qs + 1)
                    ncq = W // P
                    cps = [
                        pscp.tile([P, D], F32, tag="psc", name=f"cps{qs}_{i}")
                        for i in range(ncq)
                    ]
                    lps = [
                        pslp.tile([P, 2], F32, tag="psl", name=f"lps{qs}_{i}")
                        for i in range(ncq)
                    ]
                    for kt in range(ext_kt):
                        for qt in range(ncq):
                            ej = 2 * (qs * ncq + qt) + 2  # this position's extent
                            if kt >= ej:
                                continue
                            lhs = pTall[:, qs, kt, qt * P : (qt + 1) * P]
                            nc.tensor.matmul(
                                cps[qt][:, 0:512],
                                lhs,
                                vsb[:, kt, 0:512],
                                start=(kt == 0),
                                stop=(kt == ej - 1),
                            )
                            nc.tensor.matmul(
                                cps[qt][:, 512:1024],
                                lhs,
                                vsb[:, kt, 512:1024],
                                start=(kt == 0),
                                stop=(kt == ej - 1),
                            )
                            nc.tensor.matmul(
                                lps[qt],
                                lhs,
                                ones,
                                start=(kt == 0),
                                stop=(kt == ej - 1),
                            )
                    for qt in range(ncq):
                        qrow = q0 + qt * P
                        rt = sm.tile([P, 1], F32, tag="rt")
                        nc.vector.reciprocal(rt, lps[qt][:, 0:1])
                        ot = outp.tile([P, D], F16, tag="ot")
                        nc.vector.tensor_scalar_mul(ot, cps[qt], rt)
                        eng = nc.sync if qt == 0 else nc.scalar
                        eng.dma_start(out_d[qrow : qrow + P, :], ot)
    nc.compile()
    return nc


def _get_nc(key="f16"):
    if "nc" not in _NC_CACHE:
        _NC_CACHE["nc"] = build_nc()
    return _NC_CACHE["nc"]


def _qsel(h):
    """Query rows for core-half h: global q-tiles h, 2+h, ..., 14+h.

    Position j's tile 2j+h needs only k < (2j+h+1)*128, letting the kernel
    skip fully-masked k-tiles at compile time with a core-uniform program."""
    tiles = np.arange(8) * 2 + h
    return (tiles[:, None] * P + np.arange(P)[None, :]).reshape(-1)


def _ptile(mat):
    """[D, n] -> partition-major [128, (D/128)*n]: row p = concat over
    d-tiles a of mat[a*128+p, :]."""
    d, n = mat.shape
    return np.ascontiguousarray(
        mat.reshape(d // P, P, n).transpose(1, 0, 2).reshape(P, (d // P) * n)
    )


def _ptile_c(mat):
    """[D, 1024] -> [128, 8192] chunk-major: row p = concat over column
    halves c of (concat over d-tiles a of mat[a*128+p, c*512:(c+1)*512])."""
    d, n = mat.shape
    return np.ascontiguousarray(
        mat.reshape(d // P, P, 2, n // 2)
        .transpose(1, 2, 0, 3)
        .reshape(P, d // P * n)
    )


def make_in_maps(x, Wq, Wk, Wv):
    x = np.asarray(x, dtype=np.float32)
    Wq16 = _ptile_c(np.asarray(Wq, dtype=np.float16))
    Wk16 = _ptile_c(np.asarray(Wk, dtype=np.float16))
    Wv16 = _ptile_c(np.asarray(Wv, dtype=np.float16))
    in_maps = []
    for c in range(8):
        b, h = c // 2, c % 2
        qsel = _qsel(h)
        xb16 = x[b].astype(np.float16)
        in_maps.append(
            {
                "xkvT": _ptile_c(np.ascontiguousarray(xb16[h * HK : (h + 1) * HK].T)),
                "xqT": _ptile(np.ascontiguousarray(xb16[qsel].T)),
                "qg": qsel.astype(np.float32),
                "Wq": Wq16,
                "Wk": Wk16,
                "Wv": Wv16,
            }
        )
    return in_maps


def kernel(x, Wq, Wk, Wv, _trace=False, _nc_key="f16"):
    nc = _get_nc(_nc_key)
    in_maps = make_in_maps(x, Wq, Wk, Wv)
    res = run_bass_kernel_spmd(nc, in_maps, core_ids=list(range(8)), trace=_trace)
    out = np.empty((B, S, D), dtype=np.float32)
    for c in range(8):
        b, h = c // 2, c % 2
        out[b, _qsel(h), :] = res.results[c]["out"].astype(np.float32)
    if _trace:
        kernel.last_results = res
    return out
